# revision 7
# baseline (speedup 1.0000x reference)
"""Trainium2 Bass kernel for nn_Attention_11433202942207.

Spatial-reduction attention (PVT-style) on [B=8, N=4096, C=512]:
  q = x @ q_w.T + q_b                          (heads=8, d=64)
  x_sr = LN(conv2x2s2(x) + sr_b) * g + b      (N2=1024)
  k, v = x_sr @ kv_w.T + kv_b
  out = softmax(q k^T / sqrt(d)) v @ proj_w.T + proj_b

Distribution: data-parallel over batch, one batch element per NeuronCore
(8 cores). No collectives needed.

Device strategy (per core, all matmul inputs bf16, fp32 accumulate):
  - host pre-transposes x to xT [C, N] with tokens sigma-permuted so the
    2x2/stride-2 conv patches become single-stride access patterns.
  - phase A (PE-light): conv as matmul over K=(pixel,cin)=2048 with
    strided lhsT views of xT; LN in natural layout; transpose to x_srT via
    TensorE; kT emitted per 128-key chunk right after its transposes so
    attention score tiles can be PRE-EMITTED into the phase-A window
    (the ScalarE exp stream is the kernel's critical path; starting it
    ~35us earlier shortens the wall).
  - phase C (ACT-bound on exp, PE packed underneath): per token-block:
    qT computed just-in-time, then per head-pair: K=64 QK matmuls
    row-tiled at partition bases 0/64 (concurrent on the 16x 32x32 PE
    sub-arrays), exp on ScalarE over [128,1024] PSUM tiles,
    ones-augmented AV accumulation (denominator rides row 64) trailing by
    two kc slots, K=1 broadcast matmul + reciprocal + multiply for
    normalization.
  - attention output is assembled into a 128-partition layout aoT2
    [128(c of head pair), 4, 512]: even heads written by DVE directly at
    partitions 0:64, odd heads staged and partition-shifted 0:64 -> 64:128
    by an SBUF->SBUF DMA on the idle Pool queue. proj then contracts with
    K=128 (half the matmul streams of the K=64 per-head form).
"""

import sys

sys.path.insert(0, "/opt/trn_rl_repo")

import numpy as np

import concourse.bass as bass
from concourse import bacc, mybir
from concourse.tile import TileContext
from concourse.masks import make_identity

F32 = mybir.dt.float32
F32R = mybir.dt.float32r
BF16 = mybir.dt.bfloat16

B, N, C = 8, 4096, 512
NH, D = 8, 64
N2 = 1024
TB = 8          # token blocks of 512
NCORES = 8
LN_EPS = 1e-5
PRE_MAX = 16    # max pre-emitted (tb, hp, kc) score/exp cells
PRE_PER_STEP = 2


def _sigma_permute(x):
    """[B, 4096, C] row-major tokens -> 2x2-block-interleaved token order."""
    b = x.shape[0]
    return (
        x.reshape(b, 32, 2, 32, 2, C)
        .transpose(0, 1, 3, 2, 4, 5)
        .reshape(b, N, C)
    )


def _sigma_unpermute(y):
    b = y.shape[0]
    return (
        y.reshape(b, 32, 32, 2, 2, C)
        .transpose(0, 1, 3, 2, 4, 5)
        .reshape(b, N, C)
    )


FLAGS = {"A": True, "B": True, "C": True, "exp": True, "qk": True,
         "av": True, "norm": True, "proj": True, "dma_shift": True,
         "pre": True}


def build_nc(reps: int = 1, flags=None) -> bass.Bass:
    """Build the per-core graph. reps>1 wraps the compute body in a
    device-side For_i loop (used only for timing calibration).
    flags: ablation switches (timing experiments only)."""
    fl = dict(FLAGS)
    if flags:
        fl.update(flags)
    nc = bacc.Bacc(target_bir_lowering=False)

    xT = nc.declare_dram_parameter("xT", [C, N], BF16, isOutput=False)
    qw = nc.declare_dram_parameter("q_wT", [C, C], BF16, isOutput=False)
    qb = nc.declare_dram_parameter("q_b", [C], F32, isOutput=False)
    srw = nc.declare_dram_parameter("srw", [4 * C, C], BF16, isOutput=False)
    srb = nc.declare_dram_parameter("sr_b", [C], F32, isOutput=False)
    kvw = nc.declare_dram_parameter("kv_wT", [C, 2 * C], BF16, isOutput=False)
    kvbk = nc.declare_dram_parameter("kv_bk", [C], F32, isOutput=False)
    kvbv = nc.declare_dram_parameter("kv_bv", [C], F32, isOutput=False)
    pw = nc.declare_dram_parameter("proj_wT", [C, C], BF16, isOutput=False)
    pb = nc.declare_dram_parameter("proj_b", [C], F32, isOutput=False)
    out = nc.declare_dram_parameter("out", [N, C], F32, isOutput=True)

    def bcast_load(dst, src_handle):
        ap = src_handle[:]
        nc.gpsimd.dma_start(
            out=dst,
            in_=bass.AP(tensor=ap.tensor, offset=ap.offset, ap=[[0, 128], [1, C]]),
        )

    with nc.allow_low_precision(reason="bf16 matmul inputs; accumulation is fp32"):
        with TileContext(nc) as tc:
            # ---- persistent tiles --------------------------------------
            persist_cm = tc.tile_pool(name="persist", bufs=1)
            persist = persist_cm.__enter__()
            qT = persist.tile([128, 4, N], BF16)
            x_srT = persist.tile([128, 4, N2], BF16)      # 16KB/part
            kT = persist.tile([128, 4, N2], BF16)
            v_sb = persist.tile([128, 8, NH, 66], BF16)   # ~16.5KB/part
            pw_sb = persist.tile([128, 4, C], BF16)
            qw_sb = persist.tile([128, 4, C], BF16)
            pb_bc = persist.tile([128, C], F32)
            srb_bc = persist.tile([128, C], F32)
            kvbv_bc = persist.tile([128, C], F32)
            qb_sb = persist.tile([128, 4], F32)
            kvbk_sb = persist.tile([128, 4], F32)
            eps_sb = persist.tile([128, 1], F32)
            ones_col = persist.tile([128, D], F32R)

            nc.vector.memset(eps_sb[:], LN_EPS)
            nc.vector.memset(ones_col[:].bitcast(F32), 1.0)
            nc.vector.memset(v_sb[:, :, :, 64:66], 1.0)
            bcast_load(pb_bc[:], pb)
            bcast_load(srb_bc[:], srb)
            bcast_load(kvbv_bc[:], kvbv)
            nc.sync.dma_start(out=qb_sb[:], in_=qb[:].rearrange("(c p) -> p c", p=128))
            nc.sync.dma_start(
                out=kvbk_sb[:], in_=kvbk[:].rearrange("(c p) -> p c", p=128)
            )
            nc.sync.dma_start(
                out=pw_sb[:], in_=pw[:, :].rearrange("(c p) n -> p c n", p=128)
            )
            qw_r = qw[:, :].rearrange("(c p) n -> p c n", p=128)
            for cq in range(4):
                nc.sync.dma_start(out=qw_sb[:, cq:cq + 1, :],
                                  in_=qw_r[:, cq:cq + 1, :])

            def _emit_body():
                with tc.tile_pool(name="phC", bufs=2) as pc, \
                     tc.tile_pool(name="phC3", bufs=3) as pc3, \
                     tc.tile_pool(name="phCpre", bufs=PRE_MAX) as pcpre, \
                     tc.tile_pool(name="phCs", bufs=4) as pcs, \
                     tc.tile_pool(name="psS", bufs=2, space="PSUM") as psS:
                    xt_c = {}
                    exps_pre = {}

                    def load_xt(tb_):
                        ts_ = slice(512 * tb_, 512 * (tb_ + 1))
                        xt_tb = pcs.tile([128, 4, 512], BF16, tag="xtc")
                        xt_r = xT[:, :].rearrange(
                            "(c p) t -> p c t", p=128)[:, :, ts_]
                        nc.sync.dma_start(out=xt_tb[:], in_=xt_r)
                        xt_c[tb_] = xt_tb

                    def emit_qt(tb_, pool, tag):
                        ts_ = slice(512 * tb_, 512 * (tb_ + 1))
                        for mq in range(4):
                            pq_ = pool.tile([128, 512], F32, tag=tag)
                            for kc in range(4):
                                nc.tensor.matmul(
                                    pq_[:],
                                    qw_sb[:, kc, 128 * mq:128 * (mq + 1)],
                                    xt_c[tb_][:, kc, :],
                                    start=(kc == 0),
                                    stop=(kc == 3),
                                )
                            nc.vector.tensor_scalar_add(
                                out=qT[:, mq, ts_], in0=pq_[:],
                                scalar1=qb_sb[:, mq:mq + 1],
                            )

                    def emit_cell(t_, hp_, kc_):
                        """Pre-emit one (tb, hp, kc) QK pair + exp into the
                        phase-A window; the exp tile is retained in a
                        non-rotating pool until phase C's AV consumes it."""
                        ts_ = slice(512 * t_, 512 * (t_ + 1))
                        ps_ = psS.tile([128, 1024], F32, tag="ps_s")
                        nc.tensor.matmul(
                            ps_[:, 0:512],
                            kT[0:64, hp_, 128 * kc_:128 * (kc_ + 1)],
                            qT[0:64, hp_, ts_],
                            start=True, stop=True,
                        )
                        nc.tensor.matmul(
                            ps_[:, 512:1024],
                            kT[64:128, hp_, 128 * kc_:128 * (kc_ + 1)],
                            qT[64:128, hp_, ts_],
                            start=True, stop=True,
                        )
                        expT = pcpre.tile([128, 1024], BF16, tag="expPre")
                        nc.scalar.activation(
                            out=expT[:], in_=ps_[:],
                            func=mybir.ActivationFunctionType.Exp,
                        )
                        exps_pre[(t_, hp_, kc_)] = expT

                    pre_queue = [(t_, hp_, kc_) for kc_ in range(2)
                                 for t_ in range(2) for hp_ in range(4)]
                    pre_idx = [0]

                    def emit_pre_cells(ready_kc, budget):
                        if not (fl["pre"] and fl["C"] and fl["qk"]
                                and fl["exp"]):
                            return
                        n = 0
                        while (n < budget and pre_idx[0] < len(pre_queue)
                               and pre_idx[0] < PRE_MAX):
                            t_, hp_, kc_ = pre_queue[pre_idx[0]]
                            if kc_ > ready_kc:
                                break
                            emit_cell(t_, hp_, kc_)
                            pre_idx[0] += 1
                            n += 1

                    # ---- phase A: conv+LN -> x_srT, kT, v -----------------
                    if not fl["A"]:
                        nc.vector.memset(x_srT[:].bitcast(F32), 0.001)
                    if fl["A"]:
                      with tc.tile_pool(name="psA1", bufs=1, space="PSUM") as psA1, \
                           tc.tile_pool(name="psA2", bufs=2, space="PSUM") as psA2, \
                           tc.tile_pool(name="phA", bufs=1) as pa, \
                           tc.tile_pool(name="phA2", bufs=2) as pa2:
                          ident = pa.tile([128, 128], F32)
                          make_identity(nc, ident[:])
                          srw_sb = pa.tile([128, 16, C], BF16)
                          kvw_sb = pa.tile([128, 4, 2 * C], BF16)
                          srw_r = srw[:, :].rearrange(
                              "(pp k p) n -> p pp k n", pp=4, p=128)

                          def emit_kt_chunk(c):
                              pk = psA2.tile([128, 4, 128], F32, tag="pq")
                              for mk in range(4):
                                  for kc in range(4):
                                      nc.tensor.matmul(
                                          pk[:, mk, :],
                                          kvw_sb[:, kc, 128 * mk:128 * (mk + 1)],
                                          x_srT[:, kc, 128 * c:128 * (c + 1)],
                                          start=(kc == 0),
                                          stop=(kc == 3),
                                      )
                              for mk in range(4):
                                  nc.vector.tensor_scalar_add(
                                      out=kT[:, mk, 128 * c:128 * (c + 1)],
                                      in0=pk[:, mk, :],
                                      scalar1=kvbk_sb[:, mk:mk + 1],
                                  )

                          def emit_v(mv_):
                              pv = psA1.tile([128, 512], F32, tag="pxsr")
                              for kc in range(4):
                                  nc.tensor.matmul(
                                      pv[:],
                                      x_srT[:, kc, 128 * mv_:128 * (mv_ + 1)],
                                      kvw_sb[:, kc, C:2 * C],
                                      start=(kc == 0),
                                      stop=(kc == 3),
                                  )
                              nc.vector.tensor_add(
                                  out=v_sb[:, mv_, :, 0:64],
                                  in0=pv[:].rearrange("p (h d) -> p h d", h=NH),
                                  in1=kvbv_bc[:, :].rearrange(
                                      "p (h d) -> p h d", h=NH),
                              )

                          prev_xsrn = [None]

                          def emit_transposes(tb_prev, xsrn_prev):
                              ptr = psA1.tile([128, 4, 128], F32, tag="ptr")
                              for cb in range(4):
                                  nc.tensor.transpose(
                                      ptr[:, cb, :],
                                      xsrn_prev[:, 128 * cb:128 * (cb + 1)],
                                      ident[:]
                                  )
                              for cb in range(4):
                                  nc.vector.tensor_copy(
                                      x_srT[:, cb,
                                            128 * tb_prev:128 * (tb_prev + 1)],
                                      ptr[:, cb, :]
                                  )

                          load_xt(0)
                          if fl["C"]:
                              emit_qt(0, psA2, "pq")

                          for tb in range(TB):
                              xt_tb = pa2.tile([128, 4, 512], BF16, tag="xt")
                              ts = slice(512 * tb, 512 * (tb + 1))
                              xt_r = xT[:, :].rearrange(
                                  "(c p) t -> p c t", p=128)[:, :, ts]
                              if tb == 0:
                                  # interleave so the first conv matmul's
                                  # inputs land first in the DMA queue
                                  for cq in range(4):
                                      nc.sync.dma_start(
                                          out=xt_tb[:, cq:cq + 1, :],
                                          in_=xt_r[:, cq:cq + 1, :])
                                      nc.sync.dma_start(
                                          out=srw_sb[:, 4 * cq:4 * (cq + 1), :],
                                          in_=srw_r[:, cq, :, :])
                                  nc.sync.dma_start(
                                      out=kvw_sb[:],
                                      in_=kvw[:, :].rearrange(
                                          "(c p) n -> p c n", p=128))
                              else:
                                  nc.sync.dma_start(out=xt_tb[:], in_=xt_r)

                              # conv chunk -> x_sr natural [128 n2, C]
                              pxsr = psA1.tile([128, 512], F32, tag="pxsr")
                              for kc in range(16):
                                  p, cb = kc // 4, kc % 4
                                  lhs = xt_tb[:, cb, :]
                                  lhs = bass.AP(
                                      tensor=lhs.tensor, offset=lhs.offset + p,
                                      ap=[lhs.ap[0], [4, 128]]
                                  )
                                  nc.tensor.matmul(
                                      pxsr[:],
                                      lhs,
                                      srw_sb[:, p * 4 + cb, :],
                                      start=(kc == 0),
                                      stop=(kc == 15),
                                  )

                              # LN (DVE) emitted before the v/kt consumers of
                              # the shared psum buffers to keep DVE FIFO sane
                              xsr = pa2.tile([128, 512], F32, tag="xsr")
                              nc.vector.tensor_add(xsr[:], pxsr[:], srb_bc[:, :])
                              stats = pa2.tile([128, 6], F32, tag="stats")
                              nc.vector.bn_stats(out=stats[:], in_=xsr[:])
                              mv = pa2.tile([128, 2], F32, tag="mv")
                              nc.vector.bn_aggr(out=mv[:], in_=stats[:])
                              rstd = pa2.tile([128, 1], F32, tag="rstd")
                              nc.scalar.activation(
                                  out=rstd[:],
                                  in_=mv[:, 1:2],
                                  func=mybir.ActivationFunctionType.Sqrt,
                                  bias=eps_sb[:],
                                  scale=1.0,
                              )
                              nc.vector.reciprocal(rstd[:], rstd[:])
                              xsrn = pa2.tile([128, 512], F32, tag="xsrn")
                              nc.vector.tensor_scalar(
                                  out=xsrn[:],
                                  in0=xsr[:],
                                  scalar1=mv[:, 0:1],
                                  scalar2=rstd[:],
                                  op0=mybir.AluOpType.subtract,
                                  op1=mybir.AluOpType.mult,
                              )

                              # transposes/kv of the PREVIOUS tb keep the PE
                              # busy while this tb's LN runs on DVE/ACT
                              if prev_xsrn[0] is not None:
                                  emit_transposes(tb - 1, prev_xsrn[0])
                                  if fl["B"]:
                                      emit_v(tb - 1)
                                      emit_kt_chunk(tb - 1)
                              prev_xsrn[0] = xsrn

                              if tb == 0:
                                  load_xt(1)
                                  if fl["C"]:
                                      emit_qt(1, psA2, "pq")
                              if fl["B"]:
                                  emit_pre_cells(ready_kc=tb - 2,
                                                 budget=PRE_PER_STEP)

                          emit_transposes(TB - 1, prev_xsrn[0])
                          if fl["B"]:
                              emit_v(TB - 1)
                              emit_kt_chunk(TB - 1)

                    # ---- phase C: qT (just-in-time), attention + proj -----
                    # Head pairs (2hp, 2hp+1) share kT/qT partition chunk hp
                    # at bases 0/64; the two K=64 QK matmuls per kc run
                    # concurrently in the upper/lower PE row-groups. One
                    # [128,1024] score PSUM tile per kc holds both heads; exp
                    # covers both in one ACT op. AV is ones-augmented
                    # (denominator lands in row 64) and trails QK by two kc
                    # slots. Norms and proj are deferred into the next
                    # pair/tb so the PE never waits.
                    if fl["C"]:
                      with tc.tile_pool(name="psAV", bufs=1, space="PSUM") as psAV, \
                           tc.tile_pool(name="psO", bufs=1, space="PSUM") as psO:
                        const_exp = None
                        if not fl["exp"] or not fl["qk"]:
                            const_exp = pc.tile([128, 1024], BF16, tag="cexp")
                            nc.vector.memset(const_exp[:], 0.5)

                        def emit_norm(h, pav, aoT2_):
                            # aoT2_[p, chunk, t]: chunk=h//2, partitions
                            # (h%2)*64..+64 hold head h's d rows.
                            if not fl["norm"]:
                                nc.vector.tensor_copy(
                                    aoT2_[0:64, h // 2, :], pav[0:64, :])
                                return
                            s_sb = pc.tile([128, 512], F32R, tag="s_sb")
                            nc.vector.tensor_copy(s_sb[64:65, :], pav[64:65, :])
                            pbc = psO.tile([64, 512], F32, tag="pbc")
                            nc.tensor.matmul(
                                pbc[:], ones_col[64:65, :], s_sb[64:65, :],
                                start=True, stop=True,
                            )
                            rw_sb = pc.tile([64, 512], F32, tag="rw_sb")
                            nc.vector.reciprocal_approx_fast(
                                out=rw_sb[:], in_=pbc[:])
                            if h % 2 == 0 or not fl["dma_shift"]:
                                nc.vector.tensor_mul(
                                    aoT2_[0:64, h // 2, :], pav[0:64, :],
                                    rw_sb[:])
                            else:
                                stg = pcs.tile([64, 512], BF16, tag="stg")
                                nc.vector.tensor_mul(
                                    stg[:], pav[0:64, :], rw_sb[:])
                                nc.gpsimd.dma_start(
                                    out=aoT2_[64:128, h // 2, :], in_=stg[:])

                        def emit_proj(tb_, aoT2_):
                            for mo in range(4):
                                osb = pc.tile([128, 512], F32, tag="osb")
                                if fl["proj"]:
                                    po = psAV.tile([128, 512], F32, tag="pq2")
                                    nk = 4 if fl["dma_shift"] else 8
                                    for ch in range(nk):
                                        if fl["dma_shift"]:
                                            lhsT = aoT2_[:, ch,
                                                         128 * mo:128 * (mo + 1)]
                                            rhs = pw_sb[:, ch, :]
                                        else:
                                            lhsT = aoT2_[0:64, ch // 2,
                                                         128 * mo:128 * (mo + 1)]
                                            rhs = pw_sb[0:64, ch // 2, :]
                                        nc.tensor.matmul(
                                            po[:], lhsT, rhs,
                                            start=(ch == 0),
                                            stop=(ch == nk - 1),
                                        )
                                    nc.vector.tensor_add(
                                        osb[:], po[:], pb_bc[:, :])
                                else:
                                    nc.vector.tensor_copy(osb[:], pb_bc[:, :])
                                nc.sync.dma_start(
                                    out=out[512 * tb_ + 128 * mo:
                                            512 * tb_ + 128 * (mo + 1), :],
                                    in_=osb[:],
                                )

                        pending_pair = [None]
                        pending_last = [None]
                        pending_proj = [None]
                        for tb in range(TB):
                            ts = slice(512 * tb, 512 * (tb + 1))
                            if tb not in xt_c:
                                load_xt(tb)
                                emit_qt(tb, psAV, "pq2")
                            aoT2 = pc.tile([128, 4, 512], BF16, tag="aoT2")
                            for hp in range(4):
                                h0, h1 = 2 * hp, 2 * hp + 1
                                pav0 = psAV.tile([65, 512], F32, tag="pav0")
                                pav1 = psAV.tile([65, 512], F32, tag="pav1")
                                exps = {}
                                for kc in range(8):
                                    pre = exps_pre.pop((tb, hp, kc), None)
                                    if pre is None and fl["qk"]:
                                        ps_ = psS.tile([128, 1024], F32,
                                                       tag="ps_s")
                                        nc.tensor.matmul(
                                            ps_[:, 0:512],
                                            kT[0:64, hp, 128 * kc:128 * (kc + 1)],
                                            qT[0:64, hp, ts],
                                            start=True, stop=True,
                                        )
                                        nc.tensor.matmul(
                                            ps_[:, 512:1024],
                                            kT[64:128, hp,
                                               128 * kc:128 * (kc + 1)],
                                            qT[64:128, hp, ts],
                                            start=True, stop=True,
                                        )
                                    if kc == 0:
                                        if hp == 0 and pending_last[0] is not None:
                                            aoT2_prev, ppav0, ppav1 = \
                                                pending_last[0]
                                            emit_norm(NH - 2, ppav0, aoT2_prev)
                                            emit_norm(NH - 1, ppav1, aoT2_prev)
                                            pending_last[0] = None
                                        elif pending_pair[0] is not None:
                                            ph0, ppav0, ph1, ppav1 = \
                                                pending_pair[0]
                                            emit_norm(ph0, ppav0, aoT2)
                                            emit_norm(ph1, ppav1, aoT2)
                                            pending_pair[0] = None
                                    if kc >= 2 and fl["av"]:
                                        pe_ = exps[kc - 2]
                                        nc.tensor.matmul(
                                            pav0[:], v_sb[:, kc - 2, h0, 0:65],
                                            pe_[:, 0:512],
                                            start=(kc == 2), stop=False,
                                        )
                                        nc.tensor.matmul(
                                            pav1[:], v_sb[:, kc - 2, h1, 0:65],
                                            pe_[:, 512:1024],
                                            start=(kc == 2), stop=False,
                                        )
                                    if kc == 4 and hp == 0 and \
                                            pending_proj[0] is not None:
                                        tb_prev, aoT2_prev = pending_proj[0]
                                        emit_proj(tb_prev, aoT2_prev)
                                        pending_proj[0] = None
                                    if pre is not None:
                                        exps[kc] = pre
                                    elif fl["exp"] and fl["qk"]:
                                        expT = pc3.tile([128, 1024], BF16,
                                                        tag="expT")
                                        nc.scalar.activation(
                                            out=expT[:], in_=ps_[:],
                                            func=mybir.ActivationFunctionType.Exp,
                                        )
                                        exps[kc] = expT
                                    else:
                                        exps[kc] = const_exp
                                if fl["av"]:
                                    for kc in (6, 7):
                                        pe_ = exps[kc]
                                        nc.tensor.matmul(
                                            pav0[:], v_sb[:, kc, h0, 0:65],
                                            pe_[:, 0:512],
                                            start=False, stop=(kc == 7),
                                        )
                                        nc.tensor.matmul(
                                            pav1[:], v_sb[:, kc, h1, 0:65],
                                            pe_[:, 512:1024],
                                            start=False, stop=(kc == 7),
                                        )
                                else:
                                    pe_ = const_exp if const_exp is not None \
                                        else exps[7]
                                    nc.tensor.matmul(
                                        pav0[:], v_sb[:, 7, h0, 0:65],
                                        pe_[:, 0:512],
                                        start=True, stop=True,
                                    )
                                    nc.tensor.matmul(
                                        pav1[:], v_sb[:, 7, h1, 0:65],
                                        pe_[:, 0:512],
                                        start=True, stop=True,
                                    )
                                if hp < 3:
                                    pending_pair[0] = (h0, pav0, h1, pav1)
                                else:
                                    pending_last[0] = (aoT2, pav0, pav1)
                                    pending_proj[0] = (tb, aoT2)

                        aoT2_prev, ppav0, ppav1 = pending_last[0]
                        emit_norm(NH - 2, ppav0, aoT2_prev)
                        emit_norm(NH - 1, ppav1, aoT2_prev)
                        tb_prev, aoT2_prev = pending_proj[0]
                        emit_proj(tb_prev, aoT2_prev)

            if reps > 1:
                with tc.For_i(0, reps, 1):
                    _emit_body()
            else:
                _emit_body()

            persist_cm.__exit__(None, None, None)

    nc.compile()
    return nc


def prep_in_maps(x, q_w, q_b, kv_w, kv_b, sr_w, sr_b, ln_g, ln_b, proj_w, proj_b):
    x = np.asarray(x, np.float32)
    q_w = np.asarray(q_w, np.float32)
    q_b = np.asarray(q_b, np.float32)
    kv_w = np.asarray(kv_w, np.float32)
    kv_b = np.asarray(kv_b, np.float32)
    sr_w = np.asarray(sr_w, np.float32)
    sr_b = np.asarray(sr_b, np.float32)
    ln_g = np.asarray(ln_g, np.float32)
    ln_b = np.asarray(ln_b, np.float32)
    proj_w = np.asarray(proj_w, np.float32)
    proj_b = np.asarray(proj_b, np.float32)

    import ml_dtypes
    scale = float(D) ** -0.5
    xT = np.ascontiguousarray(
        _sigma_permute(x).transpose(0, 2, 1)).astype(ml_dtypes.bfloat16)
    q_wT = np.ascontiguousarray((q_w * scale).T).astype(ml_dtypes.bfloat16)
    q_bs = (q_b * scale).astype(np.float32)
    srw = np.ascontiguousarray(
        np.transpose(sr_w, (2, 3, 1, 0)).reshape(4 * C, C)).astype(ml_dtypes.bfloat16)
    kv_w_eff = kv_w * ln_g[None, :]
    kv_b_eff = (kv_b + kv_w @ ln_b).astype(np.float32)
    kv_wT = np.ascontiguousarray(kv_w_eff.T).astype(ml_dtypes.bfloat16)
    proj_wT = np.ascontiguousarray(proj_w.T).astype(ml_dtypes.bfloat16)

    shared = {
        "q_wT": q_wT, "q_b": q_bs, "srw": srw, "sr_b": sr_b,
        "kv_wT": kv_wT, "kv_bk": kv_b_eff[:C], "kv_bv": kv_b_eff[C:],
        "proj_wT": proj_wT, "proj_b": proj_b,
    }
    return [dict(shared, xT=np.ascontiguousarray(xT[i])) for i in range(NCORES)]


_CACHED = {}


def _get_nc():
    if "nc" not in _CACHED:
        _CACHED["nc"] = build_nc()
    return _CACHED["nc"]


def kernel(x, q_w, q_b, kv_w, kv_b, sr_w, sr_b, ln_g, ln_b, proj_w, proj_b,
           H=64, W=64):
    from concourse.bass_utils import run_bass_kernel_spmd

    nc = _get_nc()
    in_maps = prep_in_maps(x, q_w, q_b, kv_w, kv_b, sr_w, sr_b, ln_g, ln_b,
                           proj_w, proj_b)
    res = run_bass_kernel_spmd(nc, in_maps, list(range(NCORES)), trace=False)
    out_perm = np.stack([res.results[i]["out"] for i in range(NCORES)], axis=0)
    return _sigma_unpermute(out_perm).astype(np.float32)
